# revision 7
# baseline (speedup 1.0000x reference)
"""CodeGen-style attention block, tensor-parallel over heads on 8 Trainium2 cores.

Strategy (megatron-style):
  - Each core owns 2 of the 16 heads: computes Q/K/V projections for its
    head-slice of w_qkv, runs causal attention for those heads, then applies
    its row-slice of w_out, producing a partial [tokens, H] output.
  - Host sums the 8 partial outputs (the out-proj contraction over heads).

On-chip layout choices:
  - Everything is computed in the "transposed" domain: qT/kT [d, token] come
    straight out of the projection (weights stationary, hidden^T moving), so
    the score matmul scoresT[k, q] = kT_chunk.T @ qT needs no transposes.
  - Softmax runs along the partition (k) axis: exp on ScalarE (mask folded in
    via additive tiles + per-key bias), the denominator via a ones-column
    matmul, normalization deferred to after A@V (per-q reciprocal broadcast
    with a K=1 ones matmul).
  - V is produced transposed like q/k, then flipped to [k, d] with PE
    transposes (needed as the stationary side of A@V).
  - Matmuls use float32r (~1e-4 rel err, ~bf16 speed at N>=256).
  - Causal block-skipping: score/AV work for fully-masked k-chunks is skipped.
"""

from contextlib import ExitStack

import numpy as np

import concourse.bacc as bacc
import concourse.mybir as mybir
import concourse.tile as tile
from concourse.bass_utils import run_bass_kernel_spmd

F32 = mybir.dt.float32
F32R = mybir.dt.float32r
AF = mybir.ActivationFunctionType

B, S, H = 2, 2048, 4096
N_HEAD, HEAD_DIM, ROT = 16, 256, 64
MAX_POS = 2048
TOK = B * S            # 4096
N_CORES = 8
HPC = N_HEAD // N_CORES  # heads per core = 2
DPC = HPC * HEAD_DIM     # dims per core = 512
NEG = -30000.0

LAST_EXEC_NS = None
_NC_CACHE = []


def _build():
    nc = bacc.Bacc("TRN2", target_bir_lowering=False, debug=False,
                   num_devices=N_CORES)

    # [w, p, hc*256+t]: hsT tiles, per-partition-contiguous
    hst_d = nc.dram_tensor("hst", [16, 128, 32 * 256], F32R, kind="ExternalInput")
    # [oc, p, hc*128+d]: per-core w_qkv column-chunks (q0..3 k0..3 v0..3)
    wqkv_d = nc.dram_tensor("wqkv", [12, 128, 32 * 128], F32R, kind="ExternalInput")
    # [p, c, n]: per-core w_out row-slice
    wout_d = nc.dram_tensor("wout", [128, 4, H], F32R, kind="ExternalInput")
    rope_d = nc.dram_tensor("rope", [128, TOK], F32, kind="ExternalInput")
    rt_d = nc.dram_tensor("rt", [64, 64], F32R, kind="ExternalInput")
    id_d = nc.dram_tensor("ident", [128, 128], F32, kind="ExternalInput")
    onc_d = nc.dram_tensor("onesc", [128, 1], F32R, kind="ExternalInput")
    onr_d = nc.dram_tensor("onesr", [1, 128], F32R, kind="ExternalInput")
    msk_d = nc.dram_tensor("masks", [128, 4, 512], F32, kind="ExternalInput")
    kb_d = nc.dram_tensor("kb", [128, 32], F32, kind="ExternalInput")
    out_d = nc.dram_tensor("out", [TOK, H], F32, kind="ExternalOutput")

    with tile.TileContext(nc) as tc:
        with tc.tile_pool(name="dram", bufs=1, space="DRAM") as dram_pool:
            qkvT = dram_pool.tile([12, 128, TOK], F32R)

            # ---------------- Phase 1: QKV projection + rotary ----------------
            with ExitStack() as st1:
                ec = st1.enter_context
                cpool = ec(tc.tile_pool(name="p1c", bufs=1))
                wpool = ec(tc.tile_pool(name="w", bufs=1))
                hpool = ec(tc.tile_pool(name="ht", bufs=2))
                spool = ec(tc.tile_pool(name="stage", bufs=6))
                tpool = ec(tc.tile_pool(name="rott", bufs=4))
                apool = ec(tc.tile_pool(name="acc", bufs=4, space="PSUM"))
                rpool = ec(tc.tile_pool(name="rp", bufs=2, space="PSUM"))
                rope_sb = cpool.tile([128, TOK], F32)
                nc.sync.dma_start(rope_sb[:], rope_d[:])
                rt_sb = cpool.tile([64, 64], F32R)
                nc.sync.dma_start(rt_sb[:], rt_d[:])

                for p in range(2):
                    wts = []
                    for j in range(6):
                        oc = p * 6 + j
                        wt = wpool.tile([128, 32 * 128], F32R, tag=f"w{j}")
                        nc.sync.dma_start(wt[:], wqkv_d[oc])
                        wts.append(wt)
                    for w in range(16):
                        ht = hpool.tile([128, 32 * 256], F32R)
                        nc.sync.dma_start(ht[:], hst_d[w])
                        ws = slice(w * 256, (w + 1) * 256)
                        for j in range(6):
                            oc = p * 6 + j
                            acc = apool.tile([128, 256], F32)
                            for hc in range(32):
                                nc.tensor.matmul(
                                    acc[:],
                                    wts[j][:, hc * 128:(hc + 1) * 128],
                                    ht[:, hc * 256:(hc + 1) * 256],
                                    start=(hc == 0), stop=(hc == 31),
                                )
                            stage = spool.tile([128, 256], F32R)
                            nc.vector.tensor_copy(stage[:], acc[:])
                            if oc in (0, 2, 4, 6):
                                # partial rotary on first 64 dims of this head
                                rp = rpool.tile([64, 256], F32)
                                nc.tensor.matmul(rp[:], rt_sb[:], stage[0:64, :])
                                t1 = tpool.tile([64, 256], F32, tag="t1")
                                nc.vector.tensor_mul(
                                    t1[:], acc[0:64, :], rope_sb[0:64, ws])
                                t2 = tpool.tile([64, 256], F32, tag="t2")
                                nc.vector.tensor_mul(
                                    t2[:], rp[:], rope_sb[64:128, ws])
                                nc.vector.tensor_add(stage[0:64, :], t1[:], t2[:])
                            nc.sync.dma_start(qkvT[oc, :, ws], stage[:])

            # ---------------- Phase 2: attention + out-proj ----------------
            with ExitStack() as st2:
                ec = st2.enter_context
                c2 = ec(tc.tile_pool(name="p2c", bufs=1))
                kpool = ec(tc.tile_pool(name="kt", bufs=1))
                vtpool = ec(tc.tile_pool(name="vt", bufs=2))
                vhpool = ec(tc.tile_pool(name="vh", bufs=1))
                qpool = ec(tc.tile_pool(name="qq", bufs=2))
                expool = ec(tc.tile_pool(name="ex", bufs=4))
                recpool = ec(tc.tile_pool(name="rec", bufs=2))
                bcspool = ec(tc.tile_pool(name="bcs", bufs=2))
                aopool = ec(tc.tile_pool(name="ao", bufs=2))
                ospool = ec(tc.tile_pool(name="os", bufs=3))
                scpool = ec(tc.tile_pool(name="sc", bufs=2, space="PSUM"))
                avpool = ec(tc.tile_pool(name="av", bufs=1, space="PSUM"))
                denpool = ec(tc.tile_pool(name="den", bufs=1, space="PSUM"))
                tppool = ec(tc.tile_pool(name="tp", bufs=1, space="PSUM"))
                oppool = ec(tc.tile_pool(name="op", bufs=2, space="PSUM"))
                wout_sb = c2.tile([128, 4, H], F32R)
                nc.sync.dma_start(wout_sb[:], wout_d[:])
                msk_sb = c2.tile([128, 4, 512], F32)
                nc.sync.dma_start(msk_sb[:], msk_d[:])
                kb_sb = c2.tile([128, 32], F32)
                nc.sync.dma_start(kb_sb[:], kb_d[:])
                id_sb = c2.tile([128, 128], F32)
                nc.sync.dma_start(id_sb[:], id_d[:])
                onc_sb = c2.tile([128, 1], F32R)
                nc.sync.dma_start(onc_sb[:], onc_d[:])
                onr_sb = c2.tile([1, 128], F32R)
                nc.sync.dma_start(onr_sb[:], onr_d[:])

                for b in range(2):
                    bs = slice(b * 2048, (b + 1) * 2048)
                    kts = {}
                    vhs = {}
                    for hl in range(2):
                        for dc in range(2):
                            kt = kpool.tile([128, 2048], F32R, tag=f"kt{hl}{dc}")
                            nc.sync.dma_start(kt[:], qkvT[4 + 2 * hl + dc, :, bs])
                            kts[(hl, dc)] = kt
                        vh = vhpool.tile([128, 16 * 256], F32R, tag=f"vh{hl}")
                        for dc in range(2):
                            vt = vtpool.tile([128, 2048], F32)
                            nc.sync.dma_start(
                                vt[:], qkvT[8 + 2 * hl + dc, :, bs].bitcast(F32))
                            for kc in range(16):
                                tp = tppool.tile([128, 128], F32)
                                nc.tensor.transpose(
                                    tp[:], vt[:, kc * 128:(kc + 1) * 128], id_sb[:])
                                nc.vector.tensor_copy(
                                    vh[:, kc * 256 + dc * 128:
                                       kc * 256 + (dc + 1) * 128], tp[:])
                        vhs[hl] = vh

                    for qt in range(4):
                        nkc = 4 * qt + 4  # causal: k-chunks beyond are all-masked
                        qs0 = b * 2048 + qt * 512
                        aos = {}
                        for hl in range(2):
                            qs = []
                            for dc in range(2):
                                q = qpool.tile([128, 512], F32R, tag=f"q{dc}")
                                nc.sync.dma_start(
                                    q[:], qkvT[2 * hl + dc, :, qs0:qs0 + 512])
                                qs.append(q)
                            av0 = avpool.tile([128, 512], F32, tag="av0")
                            av1 = avpool.tile([128, 512], F32, tag="av1")
                            den = denpool.tile([1, 512], F32)
                            for kc in range(nkc):
                                sc = scpool.tile([128, 512], F32)
                                nc.tensor.matmul(
                                    sc[:], kts[(hl, 0)][:, kc * 128:(kc + 1) * 128],
                                    qs[0][:], start=True, stop=False)
                                nc.tensor.matmul(
                                    sc[:], kts[(hl, 1)][:, kc * 128:(kc + 1) * 128],
                                    qs[1][:], start=False, stop=True)
                                if kc >= 4 * qt:
                                    nc.vector.tensor_add(
                                        sc[:], sc[:], msk_sb[:, kc - 4 * qt, :])
                                ex = expool.tile([128, 512], F32R)
                                nc.scalar.activation(
                                    ex[:], sc[:], AF.Exp, scale=1.0 / 16.0,
                                    bias=kb_sb[:, b * 16 + kc:b * 16 + kc + 1])
                                nc.tensor.matmul(
                                    av0[:], vhs[hl][:, kc * 256:kc * 256 + 128],
                                    ex[:], start=(kc == 0), stop=(kc == nkc - 1))
                                nc.tensor.matmul(
                                    av1[:], vhs[hl][:, kc * 256 + 128:kc * 256 + 256],
                                    ex[:], start=(kc == 0), stop=(kc == nkc - 1))
                                nc.tensor.matmul(
                                    den[:], onc_sb[:], ex[:],
                                    start=(kc == 0), stop=(kc == nkc - 1))
                            rec = recpool.tile([1, 512], F32R)
                            with nc.allow_low_precision(
                                    reason="f32r rounding feeds PE broadcast"):
                                nc.vector.reciprocal(rec[:], den[:])
                            bc = oppool.tile([128, 512], F32, tag="op")
                            nc.tensor.matmul(bc[:], onr_sb[:], rec[:])
                            bcs = bcspool.tile([128, 512], F32)
                            nc.vector.tensor_copy(bcs[:], bc[:])
                            for dc, av in ((0, av0), (1, av1)):
                                ao = aopool.tile([128, 512], F32R, tag=f"ao{hl}{dc}")
                                nc.vector.tensor_mul(ao[:], av[:], bcs[:])
                                aos[(hl, dc)] = ao
                        for tc_ in range(4):
                            for ht_ in range(8):
                                op = oppool.tile([128, 512], F32, tag="op")
                                for ci, (hl, dc) in enumerate(
                                        ((0, 0), (0, 1), (1, 0), (1, 1))):
                                    nc.tensor.matmul(
                                        op[:],
                                        aos[(hl, dc)][:, tc_ * 128:(tc_ + 1) * 128],
                                        wout_sb[:, 2 * hl + dc,
                                                ht_ * 512:(ht_ + 1) * 512],
                                        start=(ci == 0), stop=(ci == 3))
                                os_ = ospool.tile([128, 512], F32)
                                nc.vector.tensor_copy(os_[:], op[:])
                                r0 = qs0 + tc_ * 128
                                nc.sync.dma_start(
                                    out_d[r0:r0 + 128, ht_ * 512:(ht_ + 1) * 512],
                                    os_[:])
    nc.compile()
    return nc


def _get_nc():
    if not _NC_CACHE:
        _NC_CACHE.append(_build())
    return _NC_CACHE[0]


def _host_prep(hidden_states, position_ids, attention_mask, w_qkv, w_out):
    hid = np.ascontiguousarray(np.asarray(hidden_states, np.float32)).reshape(TOK, H)
    w_qkv = np.asarray(w_qkv, np.float32)
    w_out = np.asarray(w_out, np.float32)
    pos = np.asarray(position_ids).astype(np.int64)
    am = np.asarray(attention_mask).reshape(B, S).astype(bool)

    # hsT tiles [w, p, hc*256+t]
    hst = np.ascontiguousarray(
        hid.reshape(16, 256, 32, 128).transpose(0, 3, 2, 1)).reshape(16, 128, 32 * 256)

    # rotary tables, matching reference.create_sinusoidal_positions
    inv_freq = 1.0 / 10000 ** (np.arange(0, ROT, 2) / ROT)
    si = np.einsum('i,j->ij', np.arange(MAX_POS), inv_freq).astype('float32')
    emb = np.concatenate([np.sin(si), np.cos(si)], axis=-1)  # [2048, 64]
    sincos = emb[pos]                    # [B, S, 64]
    sin_rep = np.repeat(sincos[..., :ROT // 2], 2, axis=2)   # [B, S, 64]
    cos_rep = np.repeat(sincos[..., ROT // 2:], 2, axis=2)
    rope = np.empty((128, TOK), np.float32)
    rope[0:64] = cos_rep.reshape(TOK, 64).T
    rope[64:128] = sin_rep.reshape(TOK, 64).T

    rt = np.zeros((64, 64), np.float32)
    rt[np.arange(1, 64, 2), np.arange(0, 64, 2)] = -1.0
    rt[np.arange(0, 64, 2), np.arange(1, 64, 2)] = 1.0

    ident = np.eye(128, dtype=np.float32)
    onesc = np.ones((128, 1), np.float32)
    onesr = np.ones((1, 128), np.float32)

    p_idx = np.arange(128)[:, None, None]
    i_idx = np.arange(4)[None, :, None]
    q_idx = np.arange(512)[None, None, :]
    masks = np.where(p_idx + i_idx * 128 <= q_idx, 0.0, NEG).astype(np.float32)

    kb = np.where(am.reshape(B, 16, 128), 0.0, NEG).astype(
        np.float32).transpose(2, 0, 1).reshape(128, 32)
    kb = np.ascontiguousarray(kb)

    shared = dict(hst=hst, rope=rope, rt=rt, ident=ident, onesc=onesc,
                  onesr=onesr, masks=masks, kb=kb)

    in_maps = []
    for c in range(N_CORES):
        cols = []
        for part in (0, 2, 1):  # fused layout per mp-group is (query, value, key)
            for hl in range(HPC):
                h = HPC * c + hl
                base = (h // 4) * 3072 + part * 1024 + (h % 4) * 256
                cols.append(np.arange(base, base + 256))
        cols = np.concatenate(cols)  # [1536] = q(512) | k(512) | v(512)
        wslice = w_qkv[:, cols]      # [4096, 1536]
        wqkv_prep = np.ascontiguousarray(
            wslice.reshape(32, 128, 12, 128).transpose(2, 1, 0, 3)
        ).reshape(12, 128, 32 * 128)
        wout_prep = np.ascontiguousarray(
            w_out[c * DPC:(c + 1) * DPC, :].reshape(4, 128, H).transpose(1, 0, 2))
        in_maps.append(dict(shared, wqkv=wqkv_prep, wout=wout_prep))
    return in_maps


def kernel(hidden_states, position_ids, attention_mask, w_qkv, w_out):
    global LAST_EXEC_NS
    nc = _get_nc()
    in_maps = _host_prep(hidden_states, position_ids, attention_mask,
                         w_qkv, w_out)
    res = run_bass_kernel_spmd(nc, in_maps, core_ids=list(range(N_CORES)))
    LAST_EXEC_NS = res.exec_time_ns
    out = res.results[0]["out"].astype(np.float32)
    for c in range(1, N_CORES):
        out = out + res.results[c]["out"]
    return out.reshape(B, S, H)


# revision 13
# speedup vs baseline: 1.0032x; 1.0032x over previous
"""CodeGen-style attention block, tensor-parallel over heads on 8 Trainium2 cores.

Strategy (megatron-style):
  - Each core owns 2 of the 16 heads: computes Q/K/V projections for its
    head-slice of w_qkv, runs causal attention for those heads, then applies
    its row-slice of w_out, producing a partial [tokens, H] output.
  - Host sums the 8 partial outputs (the out-proj contraction over heads).

On-chip layout choices:
  - Everything is computed in the "transposed" domain: qT/kT [d, token] come
    straight out of the projection (weights stationary, hidden^T moving), so
    the score matmul scoresT[k, q] = kT_chunk.T @ qT needs no transposes.
  - Softmax runs along the partition (k) axis: exp on ScalarE (mask folded in
    via additive tiles + per-key bias), the denominator via a ones-column
    matmul, normalization deferred to after A@V (per-q reciprocal broadcast
    with a K=1 ones matmul).
  - V is produced transposed like q/k, then flipped to [k, d] with PE
    transposes (needed as the stationary side of A@V).
  - Matmuls use float32r (~1e-4 rel err, ~bf16 speed at N>=256).
  - Causal block-skipping: score/AV work for fully-masked k-chunks is skipped.
"""

from contextlib import ExitStack

import numpy as np

import concourse.bacc as bacc
import concourse.mybir as mybir
import concourse.tile as tile
from concourse.bass_utils import run_bass_kernel_spmd

F32 = mybir.dt.float32
F32R = mybir.dt.float32r
AF = mybir.ActivationFunctionType

B, S, H = 2, 2048, 4096
N_HEAD, HEAD_DIM, ROT = 16, 256, 64
MAX_POS = 2048
TOK = B * S            # 4096
N_CORES = 8
HPC = N_HEAD // N_CORES  # heads per core = 2
DPC = HPC * HEAD_DIM     # dims per core = 512
NEG = -30000.0

LAST_EXEC_NS = None
_NC_CACHE = []


def _build():
    nc = bacc.Bacc("TRN2", target_bir_lowering=False, debug=False,
                   num_devices=N_CORES)

    # [w, p, hc*256+t]: hsT tiles, per-partition-contiguous
    hst_d = nc.dram_tensor("hst", [16, 128, 32 * 256], F32R, kind="ExternalInput")
    # [oc, p, hc*128+d]: per-core w_qkv column-chunks (q0..3 k0..3 v0..3)
    wqkv_d = nc.dram_tensor("wqkv", [12, 128, 32 * 128], F32R, kind="ExternalInput")
    # [p, c, n]: per-core w_out row-slice
    wout_d = nc.dram_tensor("wout", [128, 4, H], F32R, kind="ExternalInput")
    rope_d = nc.dram_tensor("rope", [128, TOK], F32, kind="ExternalInput")
    rt_d = nc.dram_tensor("rt", [64, 64], F32R, kind="ExternalInput")
    id_d = nc.dram_tensor("ident", [128, 128], F32, kind="ExternalInput")
    onm_d = nc.dram_tensor("onesm", [128, 128], F32R, kind="ExternalInput")
    msk_d = nc.dram_tensor("masks", [128, 4, 512], F32, kind="ExternalInput")
    kb_d = nc.dram_tensor("kb", [128, 32], F32, kind="ExternalInput")
    out_d = nc.dram_tensor("out", [TOK, H], F32, kind="ExternalOutput")

    # phase-1 oc order: v and k first so attention inputs for batch 0 are
    # ready while the q projections still run; q last.
    PASS_OCS = ((8, 9, 10, 11, 4, 5), (6, 7, 0, 1, 2, 3))

    with tile.TileContext(nc) as tc:
        with ExitStack() as st0:
            ec0 = st0.enter_context
            dram_pool = ec0(tc.tile_pool(name="dram", bufs=1, space="DRAM"))
            # per-(oc, batch) intermediates so phase-2 loads only wait on the
            # phase-1 windows they actually read
            qkvT = {}
            for oc in range(12):
                for b in range(2):
                    qkvT[(oc, b)] = dram_pool.tile(
                        [128, 2048], F32R, tag=f"qkvT{oc}_{b}",
                        name=f"qkvT{oc}_{b}")
            # small phase-2 constants loaded up-front (DMA is idle-ish early)
            c2 = ec0(tc.tile_pool(name="p2c", bufs=1))
            msk_sb = c2.tile([128, 4, 512], F32)
            nc.sync.dma_start(msk_sb[:], msk_d[:])
            kb_sb = c2.tile([128, 32], F32)
            nc.sync.dma_start(kb_sb[:], kb_d[:])
            id_sb = c2.tile([128, 128], F32)
            nc.sync.dma_start(id_sb[:], id_d[:])
            onm_sb = c2.tile([128, 128], F32R)
            nc.sync.dma_start(onm_sb[:], onm_d[:])

            # ---------------- Phase 1: QKV projection + rotary ----------------
            with ExitStack() as st1:
                ec = st1.enter_context
                cpool = ec(tc.tile_pool(name="p1c", bufs=1))
                wpool = ec(tc.tile_pool(name="w", bufs=1))
                hpool = ec(tc.tile_pool(name="ht", bufs=2))
                spool = ec(tc.tile_pool(name="stage", bufs=6))
                tpool = ec(tc.tile_pool(name="rott", bufs=4))
                apool = ec(tc.tile_pool(name="acc", bufs=4, space="PSUM"))
                rpool = ec(tc.tile_pool(name="rp", bufs=2, space="PSUM"))
                rope_sb = cpool.tile([128, TOK], F32)
                rt_sb = cpool.tile([64, 64], F32R)

                first = True
                for ocs in PASS_OCS:
                    wts = []
                    for j, oc in enumerate(ocs):
                        wt = wpool.tile([128, 32 * 128], F32R, tag=f"w{j}")
                        nc.sync.dma_start(wt[:], wqkv_d[oc])
                        wts.append(wt)
                    if first:
                        # constants after the first weight tiles on purpose:
                        # the first matmuls need w0/ht0, not these
                        nc.sync.dma_start(rope_sb[:], rope_d[:])
                        nc.sync.dma_start(rt_sb[:], rt_d[:])
                        first = False
                    for w in range(16):
                        ht = hpool.tile([128, 32 * 256], F32R)
                        nc.sync.dma_start(ht[:], hst_d[w])
                        ws = slice(w * 256, (w + 1) * 256)
                        wb, wo = w // 8, (w % 8) * 256
                        for j, oc in enumerate(ocs):
                            acc = apool.tile([128, 256], F32)
                            for hc in range(32):
                                nc.tensor.matmul(
                                    acc[:],
                                    wts[j][:, hc * 128:(hc + 1) * 128],
                                    ht[:, hc * 256:(hc + 1) * 256],
                                    start=(hc == 0), stop=(hc == 31),
                                )
                            stage = spool.tile([128, 256], F32R)
                            nc.vector.tensor_copy(stage[:], acc[:])
                            if oc in (0, 2, 4, 6):
                                # partial rotary on first 64 dims of this head
                                rp = rpool.tile([64, 256], F32)
                                nc.tensor.matmul(rp[:], rt_sb[:], stage[0:64, :])
                                t1 = tpool.tile([64, 256], F32, tag="t1")
                                nc.vector.tensor_mul(
                                    t1[:], acc[0:64, :], rope_sb[0:64, ws])
                                t2 = tpool.tile([64, 256], F32, tag="t2")
                                nc.vector.tensor_mul(
                                    t2[:], rp[:], rope_sb[64:128, ws])
                                nc.vector.tensor_add(stage[0:64, :], t1[:], t2[:])
                            nc.sync.dma_start(
                                qkvT[(oc, wb)][:, wo:wo + 256], stage[:])

            # ---------------- Phase 2: attention + out-proj ----------------
            with ExitStack() as st2:
                ec = st2.enter_context
                c3 = ec(tc.tile_pool(name="p2w", bufs=1))
                kpool = ec(tc.tile_pool(name="kt", bufs=1))
                vtpool = ec(tc.tile_pool(name="vt", bufs=2))
                vhpool = ec(tc.tile_pool(name="vh", bufs=1))
                qpool = ec(tc.tile_pool(name="qq", bufs=2))
                expool = ec(tc.tile_pool(name="ex", bufs=4))
                recpool = ec(tc.tile_pool(name="rec", bufs=2))
                aopool = ec(tc.tile_pool(name="ao", bufs=2))
                ospool = ec(tc.tile_pool(name="os", bufs=3))
                scpool = ec(tc.tile_pool(name="sc", bufs=2, space="PSUM"))
                avpool = ec(tc.tile_pool(name="av", bufs=1, space="PSUM"))
                denpool = ec(tc.tile_pool(name="den", bufs=2, space="PSUM"))
                oppool = ec(tc.tile_pool(name="op", bufs=2, space="PSUM"))
                wout_sb = c3.tile([128, 4, H], F32R)
                nc.sync.dma_start(wout_sb[:], wout_d[:])

                for b in range(2):
                    kts = {}
                    vhs = {}
                    for hl in range(2):
                        for dc in range(2):
                            kt = kpool.tile([128, 2048], F32R, tag=f"kt{hl}{dc}")
                            nc.sync.dma_start(kt[:], qkvT[(4 + 2 * hl + dc, b)][:])
                            kts[(hl, dc)] = kt
                        vh = vhpool.tile([128, 16 * 256], F32R, tag=f"vh{hl}")
                        for dc in range(2):
                            vt = vtpool.tile([128, 2048], F32)
                            nc.sync.dma_start(
                                vt[:], qkvT[(8 + 2 * hl + dc, b)][:].bitcast(F32))
                            for kc in range(16):
                                tp = oppool.tile([128, 128], F32, tag="op")
                                nc.tensor.transpose(
                                    tp[:], vt[:, kc * 128:(kc + 1) * 128], id_sb[:])
                                nc.vector.tensor_copy(
                                    vh[:, kc * 256 + dc * 128:
                                       kc * 256 + (dc + 1) * 128], tp[:])
                        vhs[hl] = vh

                    for qt in range(4):
                        nkc = 4 * qt + 4  # causal: k-chunks beyond are all-masked
                        qo = qt * 512
                        aos = {}
                        for hl in range(2):
                            qs = []
                            for dc in range(2):
                                q = qpool.tile([128, 512], F32R, tag=f"q{dc}")
                                nc.sync.dma_start(
                                    q[:], qkvT[(2 * hl + dc, b)][:, qo:qo + 512])
                                qs.append(q)
                            av0 = avpool.tile([128, 512], F32, tag="av0")
                            av1 = avpool.tile([128, 512], F32, tag="av1")
                            den = denpool.tile([128, 512], F32)
                            for kc in range(nkc):
                                sc = scpool.tile([128, 512], F32)
                                nc.tensor.matmul(
                                    sc[:], kts[(hl, 0)][:, kc * 128:(kc + 1) * 128],
                                    qs[0][:], start=True, stop=False)
                                nc.tensor.matmul(
                                    sc[:], kts[(hl, 1)][:, kc * 128:(kc + 1) * 128],
                                    qs[1][:], start=False, stop=True)
                                if kc >= 4 * qt:
                                    nc.vector.tensor_add(
                                        sc[:], sc[:], msk_sb[:, kc - 4 * qt, :])
                                ex = expool.tile([128, 512], F32R)
                                nc.scalar.activation(
                                    ex[:], sc[:], AF.Exp, scale=1.0 / 16.0,
                                    bias=kb_sb[:, b * 16 + kc:b * 16 + kc + 1])
                                nc.tensor.matmul(
                                    av0[:], vhs[hl][:, kc * 256:kc * 256 + 128],
                                    ex[:], start=(kc == 0), stop=(kc == nkc - 1))
                                nc.tensor.matmul(
                                    av1[:], vhs[hl][:, kc * 256 + 128:kc * 256 + 256],
                                    ex[:], start=(kc == 0), stop=(kc == nkc - 1))
                                # denominator, pre-broadcast across partitions:
                                # ones[128,128].T @ ex = colsum replicated 128x
                                nc.tensor.matmul(
                                    den[:], onm_sb[:], ex[:],
                                    start=(kc == 0), stop=(kc == nkc - 1))
                            rec = recpool.tile([128, 512], F32)
                            nc.vector.reciprocal(rec[:], den[:])
                            for dc, av in ((0, av0), (1, av1)):
                                ao = aopool.tile([128, 512], F32R, tag=f"ao{hl}{dc}")
                                nc.vector.tensor_mul(ao[:], av[:], rec[:])
                                aos[(hl, dc)] = ao
                        for tc_ in range(4):
                            for ht_ in range(8):
                                op = oppool.tile([128, 512], F32, tag="op")
                                for ci, (hl, dc) in enumerate(
                                        ((0, 0), (0, 1), (1, 0), (1, 1))):
                                    nc.tensor.matmul(
                                        op[:],
                                        aos[(hl, dc)][:, tc_ * 128:(tc_ + 1) * 128],
                                        wout_sb[:, 2 * hl + dc,
                                                ht_ * 512:(ht_ + 1) * 512],
                                        start=(ci == 0), stop=(ci == 3))
                                os_ = ospool.tile([128, 512], F32)
                                nc.vector.tensor_copy(os_[:], op[:])
                                r0 = b * 2048 + qo + tc_ * 128
                                nc.sync.dma_start(
                                    out_d[r0:r0 + 128, ht_ * 512:(ht_ + 1) * 512],
                                    os_[:])
    nc.compile()
    return nc


def _get_nc():
    if not _NC_CACHE:
        _NC_CACHE.append(_build())
    return _NC_CACHE[0]


def _host_prep(hidden_states, position_ids, attention_mask, w_qkv, w_out):
    hid = np.ascontiguousarray(np.asarray(hidden_states, np.float32)).reshape(TOK, H)
    w_qkv = np.asarray(w_qkv, np.float32)
    w_out = np.asarray(w_out, np.float32)
    pos = np.asarray(position_ids).astype(np.int64)
    am = np.asarray(attention_mask).reshape(B, S).astype(bool)

    # hsT tiles [w, p, hc*256+t]
    hst = np.ascontiguousarray(
        hid.reshape(16, 256, 32, 128).transpose(0, 3, 2, 1)).reshape(16, 128, 32 * 256)

    # rotary tables, matching reference.create_sinusoidal_positions
    inv_freq = 1.0 / 10000 ** (np.arange(0, ROT, 2) / ROT)
    si = np.einsum('i,j->ij', np.arange(MAX_POS), inv_freq).astype('float32')
    emb = np.concatenate([np.sin(si), np.cos(si)], axis=-1)  # [2048, 64]
    sincos = emb[pos]                    # [B, S, 64]
    sin_rep = np.repeat(sincos[..., :ROT // 2], 2, axis=2)   # [B, S, 64]
    cos_rep = np.repeat(sincos[..., ROT // 2:], 2, axis=2)
    rope = np.empty((128, TOK), np.float32)
    rope[0:64] = cos_rep.reshape(TOK, 64).T
    rope[64:128] = sin_rep.reshape(TOK, 64).T

    rt = np.zeros((64, 64), np.float32)
    rt[np.arange(1, 64, 2), np.arange(0, 64, 2)] = -1.0
    rt[np.arange(0, 64, 2), np.arange(1, 64, 2)] = 1.0

    ident = np.eye(128, dtype=np.float32)
    onesm = np.ones((128, 128), np.float32)

    p_idx = np.arange(128)[:, None, None]
    i_idx = np.arange(4)[None, :, None]
    q_idx = np.arange(512)[None, None, :]
    masks = np.where(p_idx + i_idx * 128 <= q_idx, 0.0, NEG).astype(np.float32)

    kb = np.where(am.reshape(B, 16, 128), 0.0, NEG).astype(
        np.float32).transpose(2, 0, 1).reshape(128, 32)
    kb = np.ascontiguousarray(kb)

    shared = dict(hst=hst, rope=rope, rt=rt, ident=ident, onesm=onesm,
                  masks=masks, kb=kb)

    in_maps = []
    for c in range(N_CORES):
        cols = []
        for part in (0, 2, 1):  # fused layout per mp-group is (query, value, key)
            for hl in range(HPC):
                h = HPC * c + hl
                base = (h // 4) * 3072 + part * 1024 + (h % 4) * 256
                cols.append(np.arange(base, base + 256))
        cols = np.concatenate(cols)  # [1536] = q(512) | k(512) | v(512)
        wslice = w_qkv[:, cols]      # [4096, 1536]
        wqkv_prep = np.ascontiguousarray(
            wslice.reshape(32, 128, 12, 128).transpose(2, 1, 0, 3)
        ).reshape(12, 128, 32 * 128)
        wout_prep = np.ascontiguousarray(
            w_out[c * DPC:(c + 1) * DPC, :].reshape(4, 128, H).transpose(1, 0, 2))
        in_maps.append(dict(shared, wqkv=wqkv_prep, wout=wout_prep))
    return in_maps


def kernel(hidden_states, position_ids, attention_mask, w_qkv, w_out):
    global LAST_EXEC_NS
    nc = _get_nc()
    in_maps = _host_prep(hidden_states, position_ids, attention_mask,
                         w_qkv, w_out)
    res = run_bass_kernel_spmd(nc, in_maps, core_ids=list(range(N_CORES)))
    LAST_EXEC_NS = res.exec_time_ns
    out = res.results[0]["out"].astype(np.float32)
    for c in range(1, N_CORES):
        out = out + res.results[c]["out"]
    return out.reshape(B, S, H)


# revision 17
# speedup vs baseline: 1.0214x; 1.0182x over previous
"""CodeGen-style attention block, tensor-parallel over heads on 8 Trainium2 cores.

Strategy (megatron-style):
  - Each core owns 2 of the 16 heads: computes Q/K/V projections for its
    head-slice of w_qkv, runs causal attention for those heads, then applies
    its row-slice of w_out, producing a partial [tokens, H] output.
  - Host sums the 8 partial outputs (the out-proj contraction over heads).

On-chip layout choices:
  - Everything is computed in the "transposed" domain: qT/kT [d, token] come
    straight out of the projection (weights stationary, hidden^T moving), so
    the score matmul scoresT[k, q] = kT_chunk.T @ qT needs no transposes.
  - Softmax runs along the partition (k) axis: exp on ScalarE (mask folded in
    via additive tiles + per-key bias), the denominator via a ones-column
    matmul, normalization deferred to after A@V (per-q reciprocal broadcast
    with a K=1 ones matmul).
  - V is produced transposed like q/k, then flipped to [k, d] with PE
    transposes (needed as the stationary side of A@V).
  - Matmuls use float32r (~1e-4 rel err, ~bf16 speed at N>=256).
  - Causal block-skipping: score/AV work for fully-masked k-chunks is skipped.
"""

from contextlib import ExitStack

import numpy as np

import concourse.bacc as bacc
import concourse.mybir as mybir
import concourse.tile as tile
from concourse.bass_utils import run_bass_kernel_spmd

F32 = mybir.dt.float32
F32R = mybir.dt.float32r
AF = mybir.ActivationFunctionType

B, S, H = 2, 2048, 4096
N_HEAD, HEAD_DIM, ROT = 16, 256, 64
MAX_POS = 2048
TOK = B * S            # 4096
N_CORES = 8
HPC = N_HEAD // N_CORES  # heads per core = 2
DPC = HPC * HEAD_DIM     # dims per core = 512
NEG = -30000.0

LAST_EXEC_NS = None
_NC_CACHE = []


def _build():
    nc = bacc.Bacc("TRN2", target_bir_lowering=False, debug=False,
                   num_devices=N_CORES)

    # [w, p, hc*256+t]: hsT tiles, per-partition-contiguous
    hst_d = nc.dram_tensor("hst", [16, 128, 32 * 256], F32R, kind="ExternalInput")
    # [oc, p, hc*128+d]: per-core w_qkv column-chunks (q0..3 k0..3 v0..3)
    wqkv_d = nc.dram_tensor("wqkv", [12, 128, 32 * 128], F32R, kind="ExternalInput")
    # [p, c, n]: per-core w_out row-slice
    wout_d = nc.dram_tensor("wout", [128, 4, H], F32R, kind="ExternalInput")
    rope_d = nc.dram_tensor("rope", [128, TOK], F32, kind="ExternalInput")
    rt_d = nc.dram_tensor("rt", [64, 64], F32R, kind="ExternalInput")
    id_d = nc.dram_tensor("ident", [128, 128], F32, kind="ExternalInput")
    onm_d = nc.dram_tensor("onesm", [128, 128], F32R, kind="ExternalInput")
    msk_d = nc.dram_tensor("masks", [128, 4, 512], F32, kind="ExternalInput")
    kb_d = nc.dram_tensor("kb", [128, 32], F32, kind="ExternalInput")
    out_d = nc.dram_tensor("out", [TOK, H], F32, kind="ExternalOutput")

    # phase-1 oc order: v and k first so attention inputs for batch 0 are
    # ready while the q projections still run; q last.
    PASS_OCS = ((8, 9, 10, 11, 4, 5), (6, 7, 0, 1, 2, 3))

    with tile.TileContext(nc) as tc:
        with ExitStack() as st0:
            ec0 = st0.enter_context
            dram_pool = ec0(tc.tile_pool(name="dram", bufs=1, space="DRAM"))
            # per-(oc, batch) intermediates so phase-2 loads only wait on the
            # phase-1 windows they actually read
            qkvT = {}
            for oc in range(12):
                for b in range(2):
                    qkvT[(oc, b)] = dram_pool.tile(
                        [128, 2048], F32R, tag=f"qkvT{oc}_{b}",
                        name=f"qkvT{oc}_{b}")
            # small phase-2 constants loaded up-front (DMA is idle-ish early)
            c2 = ec0(tc.tile_pool(name="p2c", bufs=1))
            msk_sb = c2.tile([128, 4, 512], F32)
            nc.sync.dma_start(msk_sb[:], msk_d[:])
            kb_sb = c2.tile([128, 32], F32)
            nc.sync.dma_start(kb_sb[:], kb_d[:])
            id_sb = c2.tile([128, 128], F32)
            nc.sync.dma_start(id_sb[:], id_d[:])
            onm_sb = c2.tile([128, 128], F32R)
            nc.sync.dma_start(onm_sb[:], onm_d[:])

            # ---------------- Phase 1: QKV projection + rotary ----------------
            with ExitStack() as st1:
                ec = st1.enter_context
                cpool = ec(tc.tile_pool(name="p1c", bufs=1))
                wpool = ec(tc.tile_pool(name="w", bufs=1))
                hpool = ec(tc.tile_pool(name="ht", bufs=2))
                spool = ec(tc.tile_pool(name="stage", bufs=6))
                tpool = ec(tc.tile_pool(name="rott", bufs=4))
                apool = ec(tc.tile_pool(name="acc", bufs=4, space="PSUM"))
                rpool = ec(tc.tile_pool(name="rp", bufs=2, space="PSUM"))
                rope_sb = cpool.tile([128, TOK], F32)
                rt_sb = cpool.tile([64, 64], F32R)

                def load_w(ocs):
                    wts = []
                    for j, oc in enumerate(ocs):
                        wt = wpool.tile([128, 32 * 128], F32R, tag=f"w{j}",
                                        name=f"wt{j}")
                        nc.sync.dma_start(wt[:], wqkv_d[oc])
                        wts.append(wt)
                    return wts

                wts = load_w(PASS_OCS[0])
                nc.sync.dma_start(rope_sb[:], rope_d[:])
                nc.sync.dma_start(rt_sb[:], rt_d[:])
                for p, ocs in enumerate(PASS_OCS):
                    if p > 0:
                        wts = next_wts
                    for w in range(16):
                        ht = hpool.tile([128, 32 * 256], F32R)
                        nc.sync.dma_start(ht[:], hst_d[w])
                        ws = slice(w * 256, (w + 1) * 256)
                        wb, wo = w // 8, (w % 8) * 256
                        for j, oc in enumerate(ocs):
                            acc = apool.tile([128, 256], F32)
                            for hc in range(32):
                                nc.tensor.matmul(
                                    acc[:],
                                    wts[j][:, hc * 128:(hc + 1) * 128],
                                    ht[:, hc * 256:(hc + 1) * 256],
                                    start=(hc == 0), stop=(hc == 31),
                                )
                            stage = spool.tile([128, 256], F32R)
                            nc.vector.tensor_copy(stage[:], acc[:])
                            if oc in (0, 2, 4, 6):
                                # partial rotary on first 64 dims of this head
                                rp = rpool.tile([64, 256], F32)
                                nc.tensor.matmul(rp[:], rt_sb[:], stage[0:64, :])
                                t1 = tpool.tile([64, 256], F32, tag="t1")
                                nc.vector.tensor_mul(
                                    t1[:], acc[0:64, :], rope_sb[0:64, ws])
                                t2 = tpool.tile([64, 256], F32, tag="t2")
                                nc.vector.tensor_mul(
                                    t2[:], rp[:], rope_sb[64:128, ws])
                                nc.vector.tensor_add(stage[0:64, :], t1[:], t2[:])
                            nc.sync.dma_start(
                                qkvT[(oc, wb)][:, wo:wo + 256], stage[:])
                        if w == 15 and p + 1 < len(PASS_OCS):
                            # issue next pass's weight DMAs under this
                            # window's remaining compute
                            next_wts = load_w(PASS_OCS[p + 1])

            # ---------------- Phase 2: attention + out-proj ----------------
            with ExitStack() as st2:
                ec = st2.enter_context
                c3 = ec(tc.tile_pool(name="p2w", bufs=1))
                kpool = ec(tc.tile_pool(name="kt", bufs=1))
                vtpool = ec(tc.tile_pool(name="vt", bufs=2))
                vhpool = ec(tc.tile_pool(name="vh", bufs=1))
                qpool = ec(tc.tile_pool(name="qq", bufs=2))
                expool = ec(tc.tile_pool(name="ex", bufs=4))
                recpool = ec(tc.tile_pool(name="rec", bufs=2))
                aopool = ec(tc.tile_pool(name="ao", bufs=2))
                ospool = ec(tc.tile_pool(name="os", bufs=3))
                scpool = ec(tc.tile_pool(name="sc", bufs=2, space="PSUM"))
                avpool = ec(tc.tile_pool(name="av", bufs=1, space="PSUM"))
                denpool = ec(tc.tile_pool(name="den", bufs=2, space="PSUM"))
                oppool = ec(tc.tile_pool(name="op", bufs=2, space="PSUM"))
                wout_sb = c3.tile([128, 4, H], F32R)

                def emit_outproj(b, qt, aos):
                    qo = qt * 512
                    for tc_ in range(4):
                        for ht_ in range(8):
                            op = oppool.tile([128, 512], F32, tag="op")
                            for ci, (hl, dc) in enumerate(
                                    ((0, 0), (0, 1), (1, 0), (1, 1))):
                                nc.tensor.matmul(
                                    op[:],
                                    aos[(hl, dc)][:, tc_ * 128:(tc_ + 1) * 128],
                                    wout_sb[:, 2 * hl + dc,
                                            ht_ * 512:(ht_ + 1) * 512],
                                    start=(ci == 0), stop=(ci == 3))
                            os_ = ospool.tile([128, 512], F32)
                            nc.vector.tensor_copy(os_[:], op[:])
                            r0 = b * 2048 + qo + tc_ * 128
                            nc.sync.dma_start(
                                out_d[r0:r0 + 128, ht_ * 512:(ht_ + 1) * 512],
                                os_[:])

                pending = None
                for b in range(2):
                    kts = {}
                    vhs = {}
                    for hl in range(2):
                        vh = vhpool.tile([128, 16 * 256], F32R, tag=f"vh{hl}")
                        for dc in range(2):
                            vt = vtpool.tile([128, 2048], F32)
                            nc.sync.dma_start(
                                vt[:], qkvT[(8 + 2 * hl + dc, b)][:].bitcast(F32))
                            for kc in range(16):
                                tp = oppool.tile([128, 128], F32, tag="op")
                                nc.tensor.transpose(
                                    tp[:], vt[:, kc * 128:(kc + 1) * 128], id_sb[:])
                                nc.vector.tensor_copy(
                                    vh[:, kc * 256 + dc * 128:
                                       kc * 256 + (dc + 1) * 128], tp[:])
                        vhs[hl] = vh
                        for dc in range(2):
                            kt = kpool.tile([128, 2048], F32R, tag=f"kt{hl}{dc}")
                            nc.sync.dma_start(kt[:], qkvT[(4 + 2 * hl + dc, b)][:])
                            kts[(hl, dc)] = kt
                    if b == 0:
                        # out-proj weights are first needed one qt-block in;
                        # don't let this 8MB DMA delay the attention inputs
                        nc.sync.dma_start(wout_sb[:], wout_d[:])

                    for qt in range(4):
                        nkc = 4 * qt + 4  # causal: k-chunks beyond are all-masked
                        qo = qt * 512
                        aos = {}
                        for hl in range(2):
                            qs = []
                            for dc in range(2):
                                q = qpool.tile([128, 512], F32R, tag=f"q{dc}")
                                nc.sync.dma_start(
                                    q[:], qkvT[(2 * hl + dc, b)][:, qo:qo + 512])
                                qs.append(q)
                            av0 = avpool.tile([128, 512], F32, tag="av0")
                            av1 = avpool.tile([128, 512], F32, tag="av1")
                            den = denpool.tile([128, 512], F32)
                            for kc in range(nkc):
                                sc = scpool.tile([128, 512], F32)
                                nc.tensor.matmul(
                                    sc[:], kts[(hl, 0)][:, kc * 128:(kc + 1) * 128],
                                    qs[0][:], start=True, stop=False)
                                nc.tensor.matmul(
                                    sc[:], kts[(hl, 1)][:, kc * 128:(kc + 1) * 128],
                                    qs[1][:], start=False, stop=True)
                                if kc >= 4 * qt:
                                    nc.vector.tensor_add(
                                        sc[:], sc[:], msk_sb[:, kc - 4 * qt, :])
                                ex = expool.tile([128, 512], F32R)
                                nc.scalar.activation(
                                    ex[:], sc[:], AF.Exp, scale=1.0 / 16.0,
                                    bias=kb_sb[:, b * 16 + kc:b * 16 + kc + 1])
                                nc.tensor.matmul(
                                    av0[:], vhs[hl][:, kc * 256:kc * 256 + 128],
                                    ex[:], start=(kc == 0), stop=(kc == nkc - 1))
                                nc.tensor.matmul(
                                    av1[:], vhs[hl][:, kc * 256 + 128:kc * 256 + 256],
                                    ex[:], start=(kc == 0), stop=(kc == nkc - 1))
                                # denominator, pre-broadcast across partitions:
                                # ones[128,128].T @ ex = colsum replicated 128x
                                nc.tensor.matmul(
                                    den[:], onm_sb[:], ex[:],
                                    start=(kc == 0), stop=(kc == nkc - 1))
                            rec = recpool.tile([128, 512], F32)
                            nc.vector.reciprocal(rec[:], den[:])
                            for dc, av in ((0, av0), (1, av1)):
                                ao = aopool.tile([128, 512], F32R, tag=f"ao{hl}{dc}")
                                nc.vector.tensor_mul(ao[:], av[:], rec[:])
                                aos[(hl, dc)] = ao
                        # software pipeline: emit the PREVIOUS block's out-proj
                        # here so its matmuls sit behind this block's attention
                        # in PE program order and never wait on normalization
                        if pending is not None:
                            emit_outproj(*pending)
                        pending = (b, qt, aos)
                emit_outproj(*pending)
    nc.compile()
    return nc


def _get_nc():
    if not _NC_CACHE:
        _NC_CACHE.append(_build())
    return _NC_CACHE[0]


def _host_prep(hidden_states, position_ids, attention_mask, w_qkv, w_out):
    hid = np.ascontiguousarray(np.asarray(hidden_states, np.float32)).reshape(TOK, H)
    w_qkv = np.asarray(w_qkv, np.float32)
    w_out = np.asarray(w_out, np.float32)
    pos = np.asarray(position_ids).astype(np.int64)
    am = np.asarray(attention_mask).reshape(B, S).astype(bool)

    # hsT tiles [w, p, hc*256+t]
    hst = np.ascontiguousarray(
        hid.reshape(16, 256, 32, 128).transpose(0, 3, 2, 1)).reshape(16, 128, 32 * 256)

    # rotary tables, matching reference.create_sinusoidal_positions
    inv_freq = 1.0 / 10000 ** (np.arange(0, ROT, 2) / ROT)
    si = np.einsum('i,j->ij', np.arange(MAX_POS), inv_freq).astype('float32')
    emb = np.concatenate([np.sin(si), np.cos(si)], axis=-1)  # [2048, 64]
    sincos = emb[pos]                    # [B, S, 64]
    sin_rep = np.repeat(sincos[..., :ROT // 2], 2, axis=2)   # [B, S, 64]
    cos_rep = np.repeat(sincos[..., ROT // 2:], 2, axis=2)
    rope = np.empty((128, TOK), np.float32)
    rope[0:64] = cos_rep.reshape(TOK, 64).T
    rope[64:128] = sin_rep.reshape(TOK, 64).T

    rt = np.zeros((64, 64), np.float32)
    rt[np.arange(1, 64, 2), np.arange(0, 64, 2)] = -1.0
    rt[np.arange(0, 64, 2), np.arange(1, 64, 2)] = 1.0

    ident = np.eye(128, dtype=np.float32)
    onesm = np.ones((128, 128), np.float32)

    p_idx = np.arange(128)[:, None, None]
    i_idx = np.arange(4)[None, :, None]
    q_idx = np.arange(512)[None, None, :]
    masks = np.where(p_idx + i_idx * 128 <= q_idx, 0.0, NEG).astype(np.float32)

    kb = np.where(am.reshape(B, 16, 128), 0.0, NEG).astype(
        np.float32).transpose(2, 0, 1).reshape(128, 32)
    kb = np.ascontiguousarray(kb)

    shared = dict(hst=hst, rope=rope, rt=rt, ident=ident, onesm=onesm,
                  masks=masks, kb=kb)

    in_maps = []
    for c in range(N_CORES):
        cols = []
        for part in (0, 2, 1):  # fused layout per mp-group is (query, value, key)
            for hl in range(HPC):
                h = HPC * c + hl
                base = (h // 4) * 3072 + part * 1024 + (h % 4) * 256
                cols.append(np.arange(base, base + 256))
        cols = np.concatenate(cols)  # [1536] = q(512) | k(512) | v(512)
        wslice = w_qkv[:, cols]      # [4096, 1536]
        wqkv_prep = np.ascontiguousarray(
            wslice.reshape(32, 128, 12, 128).transpose(2, 1, 0, 3)
        ).reshape(12, 128, 32 * 128)
        wout_prep = np.ascontiguousarray(
            w_out[c * DPC:(c + 1) * DPC, :].reshape(4, 128, H).transpose(1, 0, 2))
        in_maps.append(dict(shared, wqkv=wqkv_prep, wout=wout_prep))
    return in_maps


def kernel(hidden_states, position_ids, attention_mask, w_qkv, w_out):
    global LAST_EXEC_NS
    nc = _get_nc()
    in_maps = _host_prep(hidden_states, position_ids, attention_mask,
                         w_qkv, w_out)
    res = run_bass_kernel_spmd(nc, in_maps, core_ids=list(range(N_CORES)))
    LAST_EXEC_NS = res.exec_time_ns
    out = res.results[0]["out"].astype(np.float32)
    for c in range(1, N_CORES):
        out = out + res.results[c]["out"]
    return out.reshape(B, S, H)


# revision 21
# speedup vs baseline: 1.0341x; 1.0124x over previous
"""CodeGen-style attention block, tensor-parallel over heads on 8 Trainium2 cores.

Strategy (megatron-style):
  - Each core owns 2 of the 16 heads: computes Q/K/V projections for its
    head-slice of w_qkv, runs causal attention for those heads, then applies
    its row-slice of w_out, producing a partial [tokens, H] output.
  - Host sums the 8 partial outputs (the out-proj contraction over heads).

On-chip layout choices:
  - Everything is computed in the "transposed" domain: qT/kT [d, token] come
    straight out of the projection (weights stationary, hidden^T moving), so
    the score matmul scoresT[k, q] = kT_chunk.T @ qT needs no transposes.
  - Softmax runs along the partition (k) axis: exp on ScalarE (mask folded in
    via additive tiles + per-key bias), the denominator via a ones-column
    matmul, normalization deferred to after A@V (per-q reciprocal broadcast
    with a K=1 ones matmul).
  - V is produced transposed like q/k, then flipped to [k, d] with PE
    transposes (needed as the stationary side of A@V).
  - Matmuls use float32r (~1e-4 rel err, ~bf16 speed at N>=256).
  - Causal block-skipping: score/AV work for fully-masked k-chunks is skipped.
"""

from contextlib import ExitStack

import numpy as np

import concourse.bacc as bacc
import concourse.mybir as mybir
import concourse.tile as tile
from concourse.bass_utils import run_bass_kernel_spmd

F32 = mybir.dt.float32
F32R = mybir.dt.float32r
AF = mybir.ActivationFunctionType

B, S, H = 2, 2048, 4096
N_HEAD, HEAD_DIM, ROT = 16, 256, 64
MAX_POS = 2048
TOK = B * S            # 4096
N_CORES = 8
HPC = N_HEAD // N_CORES  # heads per core = 2
DPC = HPC * HEAD_DIM     # dims per core = 512
NEG = -30000.0

LAST_EXEC_NS = None
_NC_CACHE = []


def _build():
    nc = bacc.Bacc("TRN2", target_bir_lowering=False, debug=False,
                   num_devices=N_CORES)

    # [w, p, hc*256+t]: hsT tiles, per-partition-contiguous
    hst_d = nc.dram_tensor("hst", [16, 128, 32 * 256], F32R, kind="ExternalInput")
    # [oc, p, hc*128+d]: per-core w_qkv column-chunks (q0..3 k0..3 v0..3)
    wqkv_d = nc.dram_tensor("wqkv", [12, 128, 32 * 128], F32R, kind="ExternalInput")
    # [p, c, n]: per-core w_out row-slice
    wout_d = nc.dram_tensor("wout", [128, 4, H], F32R, kind="ExternalInput")
    rope_d = nc.dram_tensor("rope", [128, TOK], F32, kind="ExternalInput")
    rt_d = nc.dram_tensor("rt", [64, 64], F32R, kind="ExternalInput")
    id_d = nc.dram_tensor("ident", [128, 128], F32, kind="ExternalInput")
    onm_d = nc.dram_tensor("onesm", [128, 128], F32R, kind="ExternalInput")
    msk_d = nc.dram_tensor("masks", [128, 4, 512], F32, kind="ExternalInput")
    kb_d = nc.dram_tensor("kb", [128, 32], F32, kind="ExternalInput")
    out_d = nc.dram_tensor("out", [TOK, H], F32, kind="ExternalOutput")

    # phase-1 oc order: v and k first so attention inputs for batch 0 are
    # ready while the q projections still run; q last.
    PASS_OCS = ((8, 9, 10, 11, 4, 5), (6, 7, 0, 1, 2, 3))

    with tile.TileContext(nc) as tc:
        with ExitStack() as st0:
            ec0 = st0.enter_context
            dram_pool = ec0(tc.tile_pool(name="dram", bufs=1, space="DRAM"))
            # per-(oc, batch) intermediates so phase-2 loads only wait on the
            # phase-1 windows they actually read
            qkvT = {}
            for oc in range(12):
                for b in range(2):
                    qkvT[(oc, b)] = dram_pool.tile(
                        [128, 2048], F32R, tag=f"qkvT{oc}_{b}",
                        name=f"qkvT{oc}_{b}")
            # small phase-2 constants loaded up-front (DMA is idle-ish early)
            c2 = ec0(tc.tile_pool(name="p2c", bufs=1))
            msk_sb = c2.tile([128, 4, 512], F32)
            nc.sync.dma_start(msk_sb[:], msk_d[:])
            kb_sb = c2.tile([128, 32], F32)
            nc.sync.dma_start(kb_sb[:], kb_d[:])
            id_sb = c2.tile([128, 128], F32)
            nc.sync.dma_start(id_sb[:], id_d[:])
            onm_sb = c2.tile([128, 128], F32R)
            nc.sync.dma_start(onm_sb[:], onm_d[:])

            # ---------------- Phase 1: QKV projection + rotary ----------------
            with ExitStack() as st1:
                ec = st1.enter_context
                cpool = ec(tc.tile_pool(name="p1c", bufs=1))
                wpool = ec(tc.tile_pool(name="w", bufs=1))
                hpool = ec(tc.tile_pool(name="ht", bufs=2))
                spool = ec(tc.tile_pool(name="stage", bufs=6))
                tpool = ec(tc.tile_pool(name="rott", bufs=4))
                apool = ec(tc.tile_pool(name="acc", bufs=4, space="PSUM"))
                rpool = ec(tc.tile_pool(name="rp", bufs=2, space="PSUM"))
                rope_sb = cpool.tile([128, TOK], F32)
                rt_sb = cpool.tile([64, 64], F32R)

                def load_w(ocs, j0=0):
                    wts = []
                    for j, oc in enumerate(ocs):
                        wt = wpool.tile([128, 32 * 128], F32R, tag=f"w{j0 + j}",
                                        name=f"wt{j0 + j}")
                        nc.sync.dma_start(wt[:], wqkv_d[oc])
                        wts.append(wt)
                    return wts

                def ht_load(w, strips):
                    # strip the transfer so the first H-chunks land (and the
                    # first matmuls start) before the whole 8MB tile arrives
                    t = hpool.tile([128, 32 * 256], F32R, name="ht")
                    step = 32 // strips
                    for s in range(strips):
                        cs = slice(s * step * 256, (s + 1) * step * 256)
                        nc.sync.dma_start(t[:, cs], hst_d[w][:, cs])
                    return t

                wts = load_w(PASS_OCS[0][:1])  # w0 first: first MMs need it
                ht = ht_load(0, 4)
                wts += load_w(PASS_OCS[0][1:], j0=1)
                nc.sync.dma_start(rope_sb[:], rope_d[:])
                nc.sync.dma_start(rt_sb[:], rt_d[:])
                for p, ocs in enumerate(PASS_OCS):
                    if p > 0:
                        wts = next_wts
                        ht = next_ht
                    for w in range(16):
                        if w > 0:
                            ht = next_ht
                        ws = slice(w * 256, (w + 1) * 256)
                        wb, wo = w // 8, (w % 8) * 256
                        for j, oc in enumerate(ocs):
                            if j == 1:
                                # prefetch next window under this one's compute
                                if w < 15:
                                    next_ht = ht_load(w + 1, 1)
                                elif p + 1 < len(PASS_OCS):
                                    next_ht = ht_load(0, 2)
                            acc = apool.tile([128, 256], F32)
                            for hc in range(32):
                                nc.tensor.matmul(
                                    acc[:],
                                    wts[j][:, hc * 128:(hc + 1) * 128],
                                    ht[:, hc * 256:(hc + 1) * 256],
                                    start=(hc == 0), stop=(hc == 31),
                                )
                            stage = spool.tile([128, 256], F32R)
                            nc.vector.tensor_copy(stage[:], acc[:])
                            if oc in (0, 2, 4, 6):
                                # partial rotary on first 64 dims of this head
                                rp = rpool.tile([64, 256], F32)
                                nc.tensor.matmul(rp[:], rt_sb[:], stage[0:64, :])
                                t1 = tpool.tile([64, 256], F32, tag="t1")
                                nc.vector.tensor_mul(
                                    t1[:], acc[0:64, :], rope_sb[0:64, ws])
                                t2 = tpool.tile([64, 256], F32, tag="t2")
                                nc.vector.tensor_mul(
                                    t2[:], rp[:], rope_sb[64:128, ws])
                                nc.vector.tensor_add(stage[0:64, :], t1[:], t2[:])
                            nc.sync.dma_start(
                                qkvT[(oc, wb)][:, wo:wo + 256], stage[:])
                        if w == 15 and p + 1 < len(PASS_OCS):
                            # issue next pass's weight DMAs under this
                            # window's remaining compute
                            next_wts = load_w(PASS_OCS[p + 1])

            # ---------------- Phase 2: attention + out-proj ----------------
            with ExitStack() as st2:
                ec = st2.enter_context
                c3 = ec(tc.tile_pool(name="p2w", bufs=1))
                kpool = ec(tc.tile_pool(name="kt", bufs=1))
                vtpool = ec(tc.tile_pool(name="vt", bufs=2))
                vhpool = ec(tc.tile_pool(name="vh", bufs=1))
                qpool = ec(tc.tile_pool(name="qq", bufs=2))
                expool = ec(tc.tile_pool(name="ex", bufs=4))
                recpool = ec(tc.tile_pool(name="rec", bufs=2))
                aopool = ec(tc.tile_pool(name="ao", bufs=2))
                ospool = ec(tc.tile_pool(name="os", bufs=3))
                scpool = ec(tc.tile_pool(name="sc", bufs=2, space="PSUM"))
                avpool = ec(tc.tile_pool(name="av", bufs=1, space="PSUM"))
                denpool = ec(tc.tile_pool(name="den", bufs=2, space="PSUM"))
                oppool = ec(tc.tile_pool(name="op", bufs=2, space="PSUM"))
                wout_sb = c3.tile([128, 4, H], F32R)

                def emit_outproj(b, qt, aos):
                    qo = qt * 512
                    for tc_ in range(4):
                        for ht_ in range(8):
                            op = oppool.tile([128, 512], F32, tag="op")
                            for ci, (hl, dc) in enumerate(
                                    ((0, 0), (0, 1), (1, 0), (1, 1))):
                                nc.tensor.matmul(
                                    op[:],
                                    aos[(hl, dc)][:, tc_ * 128:(tc_ + 1) * 128],
                                    wout_sb[:, 2 * hl + dc,
                                            ht_ * 512:(ht_ + 1) * 512],
                                    start=(ci == 0), stop=(ci == 3))
                            os_ = ospool.tile([128, 512], F32)
                            nc.vector.tensor_copy(os_[:], op[:])
                            r0 = b * 2048 + qo + tc_ * 128
                            nc.sync.dma_start(
                                out_d[r0:r0 + 128, ht_ * 512:(ht_ + 1) * 512],
                                os_[:])

                pending = None
                for b in range(2):
                    kts = {}
                    vhs = {}
                    for hl in range(2):
                        vh = vhpool.tile([128, 16 * 256], F32R, tag=f"vh{hl}")
                        for dc in range(2):
                            vt = vtpool.tile([128, 2048], F32)
                            nc.sync.dma_start(
                                vt[:], qkvT[(8 + 2 * hl + dc, b)][:].bitcast(F32))
                            for kc in range(16):
                                tp = oppool.tile([128, 128], F32, tag="op")
                                nc.tensor.transpose(
                                    tp[:], vt[:, kc * 128:(kc + 1) * 128], id_sb[:])
                                nc.vector.tensor_copy(
                                    vh[:, kc * 256 + dc * 128:
                                       kc * 256 + (dc + 1) * 128], tp[:])
                        vhs[hl] = vh
                        for dc in range(2):
                            kt = kpool.tile([128, 2048], F32R, tag=f"kt{hl}{dc}")
                            nc.sync.dma_start(kt[:], qkvT[(4 + 2 * hl + dc, b)][:])
                            kts[(hl, dc)] = kt
                    if b == 0:
                        # out-proj weights are first needed one qt-block in;
                        # don't let this 8MB DMA delay the attention inputs
                        nc.sync.dma_start(wout_sb[:], wout_d[:])

                    for qt in range(4):
                        nkc = 4 * qt + 4  # causal: k-chunks beyond are all-masked
                        qo = qt * 512
                        aos = {}
                        for hl in range(2):
                            qs = []
                            for dc in range(2):
                                q = qpool.tile([128, 512], F32R, tag=f"q{dc}")
                                nc.sync.dma_start(
                                    q[:], qkvT[(2 * hl + dc, b)][:, qo:qo + 512])
                                qs.append(q)
                            av0 = avpool.tile([128, 512], F32, tag="av0")
                            av1 = avpool.tile([128, 512], F32, tag="av1")
                            den = denpool.tile([128, 512], F32)
                            for kc in range(nkc):
                                sc = scpool.tile([128, 512], F32)
                                nc.tensor.matmul(
                                    sc[:], kts[(hl, 0)][:, kc * 128:(kc + 1) * 128],
                                    qs[0][:], start=True, stop=False)
                                nc.tensor.matmul(
                                    sc[:], kts[(hl, 1)][:, kc * 128:(kc + 1) * 128],
                                    qs[1][:], start=False, stop=True)
                                if kc >= 4 * qt:
                                    nc.vector.tensor_add(
                                        sc[:], sc[:], msk_sb[:, kc - 4 * qt, :])
                                ex = expool.tile([128, 512], F32R)
                                nc.scalar.activation(
                                    ex[:], sc[:], AF.Exp, scale=1.0 / 16.0,
                                    bias=kb_sb[:, b * 16 + kc:b * 16 + kc + 1])
                                nc.tensor.matmul(
                                    av0[:], vhs[hl][:, kc * 256:kc * 256 + 128],
                                    ex[:], start=(kc == 0), stop=(kc == nkc - 1))
                                nc.tensor.matmul(
                                    av1[:], vhs[hl][:, kc * 256 + 128:kc * 256 + 256],
                                    ex[:], start=(kc == 0), stop=(kc == nkc - 1))
                                # denominator, pre-broadcast across partitions:
                                # ones[128,128].T @ ex = colsum replicated 128x
                                nc.tensor.matmul(
                                    den[:], onm_sb[:], ex[:],
                                    start=(kc == 0), stop=(kc == nkc - 1))
                            # fast PSUM evacuation: free the av/den banks in
                            # ~0.7us each instead of holding them through the
                            # ~3.4us reciprocal
                            dens = recpool.tile([128, 512], F32, tag="dens", bufs=1)
                            nc.vector.tensor_copy(dens[:], den[:])
                            avs = []
                            for dc, av in ((0, av0), (1, av1)):
                                avc = aopool.tile([128, 512], F32, bufs=1,
                                                  tag=f"avs{hl}{dc}", name="avc")
                                nc.vector.tensor_copy(avc[:], av[:])
                                avs.append(avc)
                            rec = recpool.tile([128, 512], F32, tag="rec", bufs=1)
                            nc.vector.reciprocal(rec[:], dens[:])
                            for dc in range(2):
                                ao = aopool.tile([128, 512], F32R, tag=f"ao{hl}{dc}")
                                nc.vector.tensor_mul(ao[:], avs[dc][:], rec[:])
                                aos[(hl, dc)] = ao
                        # software pipeline: emit the PREVIOUS block's out-proj
                        # here so its matmuls sit behind this block's attention
                        # in PE program order and never wait on normalization
                        if pending is not None:
                            emit_outproj(*pending)
                        pending = (b, qt, aos)
                emit_outproj(*pending)
    nc.compile()
    return nc


def _get_nc():
    if not _NC_CACHE:
        _NC_CACHE.append(_build())
    return _NC_CACHE[0]


def _host_prep(hidden_states, position_ids, attention_mask, w_qkv, w_out):
    hid = np.ascontiguousarray(np.asarray(hidden_states, np.float32)).reshape(TOK, H)
    w_qkv = np.asarray(w_qkv, np.float32)
    w_out = np.asarray(w_out, np.float32)
    pos = np.asarray(position_ids).astype(np.int64)
    am = np.asarray(attention_mask).reshape(B, S).astype(bool)

    # hsT tiles [w, p, hc*256+t]
    hst = np.ascontiguousarray(
        hid.reshape(16, 256, 32, 128).transpose(0, 3, 2, 1)).reshape(16, 128, 32 * 256)

    # rotary tables, matching reference.create_sinusoidal_positions
    inv_freq = 1.0 / 10000 ** (np.arange(0, ROT, 2) / ROT)
    si = np.einsum('i,j->ij', np.arange(MAX_POS), inv_freq).astype('float32')
    emb = np.concatenate([np.sin(si), np.cos(si)], axis=-1)  # [2048, 64]
    sincos = emb[pos]                    # [B, S, 64]
    sin_rep = np.repeat(sincos[..., :ROT // 2], 2, axis=2)   # [B, S, 64]
    cos_rep = np.repeat(sincos[..., ROT // 2:], 2, axis=2)
    rope = np.empty((128, TOK), np.float32)
    rope[0:64] = cos_rep.reshape(TOK, 64).T
    rope[64:128] = sin_rep.reshape(TOK, 64).T

    rt = np.zeros((64, 64), np.float32)
    rt[np.arange(1, 64, 2), np.arange(0, 64, 2)] = -1.0
    rt[np.arange(0, 64, 2), np.arange(1, 64, 2)] = 1.0

    ident = np.eye(128, dtype=np.float32)
    onesm = np.ones((128, 128), np.float32)

    p_idx = np.arange(128)[:, None, None]
    i_idx = np.arange(4)[None, :, None]
    q_idx = np.arange(512)[None, None, :]
    masks = np.where(p_idx + i_idx * 128 <= q_idx, 0.0, NEG).astype(np.float32)

    kb = np.where(am.reshape(B, 16, 128), 0.0, NEG).astype(
        np.float32).transpose(2, 0, 1).reshape(128, 32)
    kb = np.ascontiguousarray(kb)

    shared = dict(hst=hst, rope=rope, rt=rt, ident=ident, onesm=onesm,
                  masks=masks, kb=kb)

    in_maps = []
    for c in range(N_CORES):
        cols = []
        for part in (0, 2, 1):  # fused layout per mp-group is (query, value, key)
            for hl in range(HPC):
                h = HPC * c + hl
                base = (h // 4) * 3072 + part * 1024 + (h % 4) * 256
                cols.append(np.arange(base, base + 256))
        cols = np.concatenate(cols)  # [1536] = q(512) | k(512) | v(512)
        wslice = w_qkv[:, cols]      # [4096, 1536]
        wqkv_prep = np.ascontiguousarray(
            wslice.reshape(32, 128, 12, 128).transpose(2, 1, 0, 3)
        ).reshape(12, 128, 32 * 128)
        wout_prep = np.ascontiguousarray(
            w_out[c * DPC:(c + 1) * DPC, :].reshape(4, 128, H).transpose(1, 0, 2))
        in_maps.append(dict(shared, wqkv=wqkv_prep, wout=wout_prep))
    return in_maps


def kernel(hidden_states, position_ids, attention_mask, w_qkv, w_out):
    global LAST_EXEC_NS
    nc = _get_nc()
    in_maps = _host_prep(hidden_states, position_ids, attention_mask,
                         w_qkv, w_out)
    res = run_bass_kernel_spmd(nc, in_maps, core_ids=list(range(N_CORES)))
    LAST_EXEC_NS = res.exec_time_ns
    out = res.results[0]["out"].astype(np.float32)
    for c in range(1, N_CORES):
        out = out + res.results[c]["out"]
    return out.reshape(B, S, H)
